# revision 1
# baseline (speedup 1.0000x reference)
"""DAWNBlock MoE-routing kernel for 8 Trainium2 NeuronCores.

Reference computation (shapes hardcoded):
  x [4, 4096, 2048] -> h = x @ W_proj + b_proj          [4, 4096, 64]
  logits = h @ normalize(neuron_emb).T                  [4, 4096, 1536]
  softmax over 3 groups of 512 (C / QK / V)
  dense_g = einsum('bs,bsn->bn', importance, softmax_g) [4, 512] x3
  top-k sparsify + renormalize (k = 8 / 4 / 4 / 6)      -> [4, 4, 512]

Sharding: data-parallel over S (4096 -> 8 x 512). Each core processes
2048 tokens (all 4 batches x its S-slice), producing a partial
dense [4, 1536]. Host sums partials and does the (tiny) top-k.

v2 design (vs the 102us baseline):
  - fp16 end-to-end on the matmul path (x/W/hT/embT/exp): halves HBM
    traffic for x (8.4 MB/core), the dominant cost.  Verified on host:
    dense rel err ~1.8e-4, zero top-k support flips (tolerance 2e-2,
    first flip at ~3e-4).
  - per-group (= per-batch) software pipeline: DMA group g+1 while
    computing group g; x arrives in 4 quarter-DMAs with 4 KiB/partition
    contiguous descriptor rows.
  - PE array packing with tile_position:
      stage1: two token-halves col-tiled (0,0)/(0,64) sharing W_k
      stage2: two token-tiles row-tiled (0,0)/(64,0), K=64 each
      pool:   three softmax groups col-tiled (0,0)/(0,32)/(0,64) into
              one shared PSUM accumulator bank
    Shared-bank accumulations use memset + start=False matmuls (the
    values are zeroed, so accumulate-vs-overwrite is correct for any
    has_written state and any PE issue order).
  - softmax z-sums on DVE via tensor_scalar(mult 1.0) with fused
    accum_out (4x perf mode on contiguous fp16).
  - exp on ACT as one [128,1536] ACTIVATE per token tile, PSUM->SBUF.
PSUM budget: 2x logits [128,1536] (3 banks each) + h2 (1) + acc (1) = 8.
"""

import os
import sys

import numpy as np

for _p in ("/opt/trn_rl_repo", os.path.expanduser("~/.axon_site/_ro/trn_rl_repo")):
    if os.path.isdir(_p) and _p not in sys.path:
        sys.path.insert(0, _p)

import concourse.bass as bass
import concourse.mybir as mybir
import concourse.tile as tile
from concourse.bass_utils import run_bass_kernel_spmd


def _ensure_axon_hooks():
    """bass_utils' trace path imports antenv.axon_hooks, which this image's
    antenv stub doesn't ship. Provide it, registering the same ctypes NTFF
    hook the axon boot shim would install when the PJRT .so supports it."""
    try:
        import antenv.axon_hooks  # noqa: F401
        return
    except ImportError:
        pass
    import contextlib
    import ctypes
    import types

    import antenv

    mod = types.ModuleType("antenv.axon_hooks")
    _box = [None]
    mod.set_axon_ntff_profile_hook = lambda h: _box.__setitem__(0, h)
    mod.get_axon_ntff_profile_hook = lambda: _box[0]
    sys.modules["antenv.axon_hooks"] = mod
    antenv.axon_hooks = mod

    so_path = "/opt/axon/libaxon_pjrt.so"
    if not os.path.exists(so_path):
        return
    try:
        lib = ctypes.CDLL(so_path)
    except OSError:
        return
    if not hasattr(lib, "axon_start_nrt_profile"):
        return
    lib.axon_start_nrt_profile.argtypes = [ctypes.POINTER(ctypes.c_int64), ctypes.c_size_t]
    lib.axon_start_nrt_profile.restype = ctypes.c_int64
    lib.axon_stop_nrt_profile.argtypes = [ctypes.c_char_p]
    lib.axon_stop_nrt_profile.restype = ctypes.c_int64

    @contextlib.contextmanager
    def _hook(output_dir, device_ids):
        import jax

        jax.devices()
        if device_ids:
            ids = (ctypes.c_int64 * len(device_ids))(*device_ids)
            rc = lib.axon_start_nrt_profile(ids, len(device_ids))
        else:
            rc = lib.axon_start_nrt_profile(None, 0)
        if rc != 0:
            raise RuntimeError(f"axon_start_nrt_profile rc={rc}")
        try:
            yield
        finally:
            n = lib.axon_stop_nrt_profile(str(output_dir).encode())
            print(f"ntff profile: {n} file(s) written to {output_dir}", file=sys.stderr)

    _box[0] = _hook


B, S, D, DS = 4, 4096, 2048, 64
N_GROUP = 512
N_TOT = 3 * N_GROUP
TOPK_C, TOPK_QK, TOPK_V = 8, 4, 6
N_CORES = 8
S_SH = S // N_CORES          # 512 sequence positions per core
T = B * S_SH                 # 2048 tokens per core
KCH = D // 128               # 16 contraction chunks
N_TTILE = T // 128           # 16 token tiles of 128
F32 = mybir.dt.float32
F16 = mybir.dt.float16

LAST_RESULTS = None  # BassKernelResults of the most recent run (for test harness)


def build_nc():
    nc = bass.Bass()
    # x, quarter-major: [g*4+q][128 part][4 chunks][512 t] fp16; each
    # partition row is 4 KiB contiguous in DRAM.
    xq = nc.declare_dram_parameter("xq", [B * 4, 128, 4, 512], F16, isOutput=False)
    impT = nc.declare_dram_parameter("impT", [128, N_TTILE], F32, isOutput=False)
    Wt = nc.declare_dram_parameter("Wt", [128, KCH * DS], F16, isOutput=False)
    b2 = nc.declare_dram_parameter("b2", [128, 1], F32, isOutput=False)
    embT2 = nc.declare_dram_parameter("embT2", [128, N_TOT], F16, isOutput=False)
    zerow = nc.declare_dram_parameter("zerow", [128, 128], F16, isOutput=False)
    densep = nc.declare_dram_parameter("densep", [68, N_GROUP], F32, isOutput=True)

    EXPF = mybir.ActivationFunctionType.Exp
    MULT = mybir.AluOpType.mult

    with tile.TileContext(nc) as tc:
        with (
            tc.tile_pool(name="consts", bufs=1) as consts,
            tc.tile_pool(name="xin", bufs=2) as xin,
            tc.tile_pool(name="hTp", bufs=2) as hTp,
            tc.tile_pool(name="expp", bufs=6) as expp,
            tc.tile_pool(name="small", bufs=4) as small,
            tc.tile_pool(name="zscrp", bufs=2) as zscrp,
            tc.tile_pool(name="outp", bufs=1) as outp,
            tc.tile_pool(name="h2_pool", bufs=1, space="PSUM") as h2_pool,
            tc.tile_pool(name="lg_pool", bufs=2, space="PSUM") as lg_pool,
            tc.tile_pool(name="acc_pool", bufs=1, space="PSUM") as acc_pool,
        ):
            w_s = consts.tile([128, KCH * DS], F16)
            nc.sync.dma_start(out=w_s, in_=Wt[:])
            embT_s = consts.tile([128, N_TOT], F16)
            nc.sync.dma_start(out=embT_s, in_=embT2[:])
            b_s = consts.tile([128, 1], F32)
            nc.sync.dma_start(out=b_s, in_=b2[:])
            imp_s = consts.tile([128, N_TTILE], F32)
            nc.sync.dma_start(out=imp_s, in_=impT[:])
            zw_s = consts.tile([128, 128], F16)
            nc.sync.dma_start(out=zw_s, in_=zerow[:])

            # acc: one PSUM bank.  Pool matmuls are col-tiled at partitions
            # 0-3 / 32-35 / 64-67 for the three softmax groups; rows 8-9
            # are a scratch target for absorber matmuls (every matmul /
            # DVE tensor-scalar has a single sync-wait slot, so tiny ops
            # pre-consume second dependencies into the engine's clock).
            acc_t = acc_pool.tile([68, N_GROUP], F32)

            def absorb(dep_ap, base=8, start=False):
                nc.tensor.matmul(
                    acc_t[base:base + 2, 0:2], dep_ap[:, 0:2], dep_ap[:, 0:2],
                    start=start, stop=False, skip_group_check=True,
                )

            # Startup absorbers write rows 0-1; the gi0 zeroing matmul below
            # re-clears the bank (start=True) and overwrites rows 0-3.
            absorb(zw_s, base=0, start=True)   # absorbs the zerow DMA lane
            absorb(w_s, base=0)                # absorbs the W DMA lane
            absorb(embT_s, base=0)             # absorbs the emb DMA lane
            # Zero the acc accumulator rows with the PE, order-safe.
            for gi in range(3):
                nc.tensor.matmul(
                    acc_t[32 * gi:32 * gi + 4, :], zw_s[:, 0:4], w_s[:, 0:512],
                    start=True, stop=False,
                    tile_position=(0, 32 * gi), skip_group_check=True,
                )
            # DVE-side absorbers for the small constants.
            dve_scr = small.tile([128, 1], F32, name="dve_scr", tag="dve_scr", bufs=1)
            dve_scr2 = small.tile([128, 1], F32, name="dve_scr2", tag="dve_scr2",
                                  bufs=1)
            nc.vector.tensor_copy(out=dve_scr, in_=b_s)
            nc.vector.tensor_copy(out=dve_scr2, in_=imp_s[:, 0:1])
            # ACT-side absorber for imp (read by the scalar.mul below) and
            # the per-j lhsT store for the pool matmuls, zeroed once: only
            # column g of tile j is ever written, the rest stays zero.
            ascr0 = small.tile([1, 1], F32, name="ascr0", tag="ascr0", bufs=1)
            nc.scalar.copy(out=ascr0, in_=imp_s[0:1, 0:1])
            cball = consts.tile([128, N_TTILE, 3, 4], F16)
            nc.vector.memset(cball, 0.0)

            zmats = {}
            for g in range(B):  # each 512-token group == one batch
                # ---- x DMA in 4 quarters (4 chunks each); separate tiles
                # so each quarter's WAW wait is a single DMA lane ----
                xgq = []
                for q in range(4):
                    xt = xin.tile([128, 4, 512], F16, name=f"xg_{g}_{q}",
                                  tag=f"xg_{g % 2}_{q}", bufs=1)
                    nc.sync.dma_start(out=xt, in_=xq[g * 4 + q])
                    xgq.append(xt)
                # ---- stage 1: h2[128, 256], two col-tiled token halves ----
                # A PE-written zero tile makes the shared-bank accumulation
                # independent of the A/B issue order (values are zeroed, so
                # accumulate-vs-overwrite both give the right answer).
                h2 = h2_pool.tile([128, 256], F32, name=f"h2_{g}", tag="h2")
                nc.tensor.matmul(h2, zw_s, w_s[:, 0:256], start=True, stop=False,
                                 skip_group_check=True)
                for k in range(KCH):
                    wk = w_s[:, k * DS:(k + 1) * DS]
                    xk = xgq[k // 4][:, k % 4, :]
                    nc.tensor.matmul(
                        h2[0:64, :], wk, xk[:, 0:256],
                        start=False, stop=(k == KCH - 1),
                        tile_position=(0, 0), skip_group_check=True,
                    )
                    nc.tensor.matmul(
                        h2[64:128, :], wk, xk[:, 256:512],
                        start=False, stop=(k == KCH - 1),
                        tile_position=(0, 64), skip_group_check=True,
                    )
                hT2 = hTp.tile([128, 256], F16, name=f"hT2_{g}", tag="hT2")
                nc.vector.tensor_scalar_add(out=hT2, in0=h2, scalar1=b_s)

                for r in range(2):  # pair-rounds: tiles (g,r) and (g,r+2)
                    rnd = g * 2 + r
                    jA = g * 4 + r          # tokens [r*128, r*128+128)
                    jB = g * 4 + r + 2      # tokens [256+r*128, ...)
                    lgA = lg_pool.tile([128, 3, N_GROUP], F32, name=f"lgA_{g}_{r}",
                                       tag="lgA", bufs=1)
                    lgB = lg_pool.tile([128, 3, N_GROUP], F32, name=f"lgB_{g}_{r}",
                                       tag="lgB", bufs=1)
                    lhA = hT2[0:64, r * 128:(r + 1) * 128]
                    lhB = hT2[64:128, r * 128:(r + 1) * 128]
                    for gi in range(3):
                        nc.tensor.matmul(
                            lgA[:, gi, :], lhA,
                            embT_s[0:64, gi * N_GROUP:(gi + 1) * N_GROUP],
                            start=True, stop=True, tile_position=(0, 0),
                        )
                        nc.tensor.matmul(
                            lgB[:, gi, :], lhB,
                            embT_s[64:128, gi * N_GROUP:(gi + 1) * N_GROUP],
                            start=True, stop=True, tile_position=(64, 0),
                        )
                    # ACT-side absorber: pull the DVE tick of round rnd-3's
                    # z sums into ACT's clock, so the exp ACTIVATEs below
                    # only need their PE (logits) wait; the exp-slot WAR
                    # (6 bufs = 3 rounds back) is then already satisfied.
                    if rnd >= 3:
                        ascr = small.tile([1, 1], F32, name=f"ascr_{rnd}",
                                          tag=f"ascr_{rnd}", bufs=1)
                        nc.scalar.copy(out=ascr, in_=zmats[rnd - 3][0:1, 0:1])
                    for half, (j, lgt) in enumerate(((jA, lgA), (jB, lgB))):
                        exp_t = expp.tile([128, 3, N_GROUP], F16, name=f"exp_{j}",
                                          tag="exp")
                        nc.scalar.activation(out=exp_t, in_=lgt, func=EXPF)
                        # z = per-token sums per softmax group (DVE 4x path)
                        zmat = small.tile([128, 3], F32, name=f"z_{j}", tag=f"z_{j}",
                                          bufs=1)
                        if half == 1:
                            zmats[rnd] = zmat
                        for gi in range(3):
                            zscr = zscrp.tile([128, N_GROUP], F16,
                                              name=f"zs_{j}_{gi}", tag="zscr")
                            nc.vector.tensor_scalar(
                                out=zscr, in0=exp_t[:, gi, :],
                                scalar1=1.0, scalar2=0.0, op0=MULT,
                                op1=mybir.AluOpType.add,
                                accum_out=zmat[:, gi:gi + 1],
                            )
                        rz = small.tile([128, 3], F32, name=f"rz_{j}", tag=f"rz_{j}",
                                        bufs=1)
                        nc.vector.reciprocal(out=rz, in_=zmat)
                        # c = imp/z on ACT, so the pool matmuls' lhsT carries
                        # the newest ACT tick (which subsumes the exp write)
                        # and they need only a single sync wait.
                        nc.scalar.mul(out=cball[:, j, :, g], in_=rz,
                                      mul=imp_s[:, j:j + 1])
                        lastmm = (g == B - 1 and r == 1 and half == 1)
                        for gi in range(3):
                            nc.tensor.matmul(
                                acc_t[32 * gi:32 * gi + 4, :],
                                cball[:, j, gi, :], exp_t[:, gi, :],
                                start=False,
                                stop=lastmm,
                                tile_position=(0, 32 * gi),
                                skip_group_check=True,
                            )

            dense_s = outp.tile([68, N_GROUP], F32)
            nc.vector.memset(dense_s, 0.0)
            for gi in range(3):
                nc.vector.tensor_copy(out=dense_s[32 * gi:32 * gi + 4, :],
                                      in_=acc_t[32 * gi:32 * gi + 4, :])
            nc.gpsimd.dma_start(out=densep[:], in_=dense_s)

    _strip_self_waits(nc)
    _strip_dma_waw_waits(nc)
    _slim_tail_drain(nc)
    return nc


_SELF_SEM = {
    "InstMatmult": "PE",
    "InstLdweights": "PE",
    "InstActivation": "Activation",
    "InstTensorScalarPtr": "DVE",
    "InstTensorTensor": "DVE",
    "InstTensorReduce": "DVE",
    "InstTensorCopy": "DVE",
    "InstReciprocal": "DVE",
}


def _strip_dma_waw_waits(nc):
    """Input-DMA rewrites of a rotating SBUF slot carry both the WAR wait on
    the slot's PE readers and a WAW wait on the slot's previous DMA lane.
    The PE readers waited on that DMA lane themselves, so the PE wait
    transitively covers the DMAHW one; DMA_DIRECT2D only has one wait slot."""
    for blk in nc.m.functions[0].blocks:
        for ins in blk.instructions:
            if type(ins).__name__ != "InstDMACopy":
                continue
            si = ins.sync_info
            if not si or len(si.on_wait) <= 1:
                continue
            waits = list(si.on_wait)
            pe = [w for w in waits if w.ant_name.startswith("PE")]
            if not pe:
                continue
            keep = [w for w in waits if not w.ant_name.startswith("DMAHW")]
            assert len(keep) <= 1, (
                f"{ins.name}: waits {[w.ant_name for w in waits]}"
            )
            ins.sync_info = mybir.SyncInfo(
                on_wait=keep, on_update=list(si.on_update))


def _strip_self_waits(nc):
    """Matmul (and several other engine-op) lowerings support a single sync
    wait.  Tile emits a same-engine completion wait for PSUM/SBUF-slot WAW
    reuse on top of the real cross-engine wait; compute engines are strict
    FIFO with in-order completion (PE matmuls are pc-monotone in start AND
    end), so the same-engine wait is hardware-redundant.  Drop it when an
    instruction carries more than one wait."""
    for blk in nc.m.functions[0].blocks:
        for ins in blk.instructions:
            sem = _SELF_SEM.get(type(ins).__name__)
            if sem is None:
                continue
            si = ins.sync_info
            if not si or len(si.on_wait) <= 1:
                continue
            keep = [w for w in si.on_wait if not w.ant_name.startswith(sem)]
            assert len(keep) <= 1, (
                f"{ins.name}: {len(keep)} cross-engine waits "
                f"{[w.ant_name for w in si.on_wait]}"
            )
            ins.sync_info = mybir.SyncInfo(
                on_wait=keep, on_update=list(si.on_update))


def _slim_tail_drain(nc):
    """The TileContext tail drain carries one wait per proc, but the SP
    CTRL_NO lowering has a small wait budget.  Every HWDGE input DMA here
    has a compute consumer and the final DVE/ACT ticks are consumed by PE /
    the SWDGE output DMA, so ordering is preserved by keeping just the PE
    wait on the SP drain and moving the DMASW wait to the (wait-free) Pool
    drain ahead of the all-engine barrier."""
    blk = nc.m.functions[0].blocks[-1]
    insts = blk.instructions
    drain = insts[0]
    assert type(drain).__name__ == "InstDrain" and drain.sync_info
    waits = list(drain.sync_info.on_wait)
    if len(waits) <= 1:
        return
    keep = [w for w in waits if w.ant_name.startswith("PE")]
    sw = [w for w in waits if w.ant_name.startswith("DMASW")]
    drain.sync_info = mybir.SyncInfo(on_wait=keep, on_update=list(drain.sync_info.on_update))
    if sw:
        for ins in insts:
            if (
                type(ins).__name__ == "InstDrain"
                and ins.engine == mybir.EngineType.Pool
                and (not ins.sync_info or len(ins.sync_info.on_wait) == 0)
            ):
                upd = list(ins.sync_info.on_update) if ins.sync_info else []
                ins.sync_info = mybir.SyncInfo(on_wait=sw, on_update=upd)
                break


def _topk_sparsify(w: np.ndarray, k: int) -> np.ndarray:
    # w [B, N]: keep top-k per row, zero the rest, renormalize.
    idx = np.argpartition(-w, k - 1, axis=-1)[:, :k]
    sparse = np.zeros_like(w)
    np.put_along_axis(sparse, idx, np.take_along_axis(w, idx, axis=-1), axis=-1)
    return sparse / (sparse.sum(axis=-1, keepdims=True) + 1e-8)


def kernel(x, importance, W_proj, b_proj, neuron_emb):
    global LAST_RESULTS
    x = np.asarray(x, dtype=np.float32)
    importance = np.asarray(importance, dtype=np.float32)
    W_proj = np.asarray(W_proj, dtype=np.float32)
    b_proj = np.asarray(b_proj, dtype=np.float32)
    neuron_emb = np.asarray(neuron_emb, dtype=np.float32)

    # Replicated small weights, device-friendly layouts.
    norm = np.maximum(np.linalg.norm(neuron_emb, axis=-1, keepdims=True), 1e-12)
    embT = np.ascontiguousarray((neuron_emb / norm).T)                      # [64, 1536]
    embT2 = np.concatenate([embT, embT], axis=0).astype(np.float16)         # [128, 1536]
    Wt = np.ascontiguousarray(
        W_proj.reshape(KCH, 128, DS).transpose(1, 0, 2).reshape(128, KCH * DS)
    ).astype(np.float16)
    b2 = np.concatenate([b_proj, b_proj]).reshape(128, 1).astype(np.float32)

    in_maps = []
    for c in range(N_CORES):
        xc = x[:, c * S_SH:(c + 1) * S_SH, :]                   # [B, 512 t, 2048 d]
        # -> [g][q][128 p][4 c][512 t] with d = (q*4+c)*128 + p
        xqc = np.ascontiguousarray(
            xc.reshape(B, S_SH, 4, 4, 128).transpose(0, 2, 4, 3, 1)
        ).astype(np.float16)
        impc = importance[:, c * S_SH:(c + 1) * S_SH].reshape(T)
        impTc = np.ascontiguousarray(impc.reshape(N_TTILE, 128).T)          # [128, 16]
        in_maps.append(
            {"xq": xqc.reshape(B * 4, 128, 4, 512), "impT": impTc, "Wt": Wt,
             "b2": b2, "embT2": embT2,
             "zerow": np.zeros((128, 128), dtype=np.float16)}
        )

    _ensure_axon_hooks()
    nc = build_nc()
    try:
        res = run_bass_kernel_spmd(nc, in_maps, core_ids=list(range(N_CORES)))
    except Exception as e:  # trace/profile plumbing can fail; rerun untraced
        if os.environ.get("BASS_NEVER_TRACE") == "1":
            raise
        print(f"traced run failed ({type(e).__name__}: {e}); retrying untraced",
              file=sys.stderr)
        os.environ["BASS_NEVER_TRACE"] = "1"
        try:
            res = run_bass_kernel_spmd(nc, in_maps, core_ids=list(range(N_CORES)))
        finally:
            del os.environ["BASS_NEVER_TRACE"]
    LAST_RESULTS = res
    if getattr(res, "exec_time_ns", None) is not None:
        print(f"HW exec time: {res.exec_time_ns} ns")

    dense = np.zeros((B, N_TOT), dtype=np.float64)
    for r in res.results:
        dp = r["densep"]  # [68, 512]: rows 0-3 = C, 32-35 = QK, 64-67 = V
        dense[:, 0:512] += dp[0:4].astype(np.float64)
        dense[:, 512:1024] += dp[32:36].astype(np.float64)
        dense[:, 1024:1536] += dp[64:68].astype(np.float64)
    dense = dense.astype(np.float32)

    dense_C = dense[:, :N_GROUP]
    dense_QK = dense[:, N_GROUP:2 * N_GROUP]
    dense_V = dense[:, 2 * N_GROUP:]
    w_C = _topk_sparsify(dense_C, TOPK_C)
    w_Q = _topk_sparsify(dense_QK, TOPK_QK)
    w_K = _topk_sparsify(dense_QK, TOPK_QK)
    w_V = _topk_sparsify(dense_V, TOPK_V)
    return np.stack([w_C, w_Q, w_K, w_V], axis=0).astype(np.float32)



# revision 8
# speedup vs baseline: 1.2755x; 1.2755x over previous
"""DAWNBlock MoE-routing kernel for 8 Trainium2 NeuronCores.

Reference computation (shapes hardcoded):
  x [4, 4096, 2048] -> h = x @ W_proj + b_proj          [4, 4096, 64]
  logits = h @ normalize(neuron_emb).T                  [4, 4096, 1536]
  softmax over 3 groups of 512 (C / QK / V)
  dense_g = einsum('bs,bsn->bn', importance, softmax_g) [4, 512] x3
  top-k sparsify + renormalize (k = 8 / 4 / 4 / 6)      -> [4, 4, 512]

Sharding: data-parallel over S (4096 -> 8 x 512). Each core processes
2048 tokens (all 4 batches x its S-slice = 16 tiles of 128 tokens),
producing a partial dense [4, 1536]. Host sums partials + tiny top-k.

v3 design (vs the 80us v2): the v2 critical path was the per-tile
softmax chain exp(ACT 1.5us) -> 3x z-accum(DVE 1x, 2.3us) -> recip ->
c(ACT) -> pool, ~2.7us/tile serial, with the PE stuck at 1.2GHz (HAM
never warmed).  v3:
  - z via two fp16 tensor_tensor pairwise folds (DVE 2x_1p fast mode)
    + one small tensor_reduce: ~1.2us/tile on DVE instead of 2.3, and
    ACT does one plain 1536-wide exp (1.57us) with no accum splits.
  - c = imp * (1/z) moved ACT -> DVE (tiny).
  - PE warm-up: junk matmuls at t=0 while DMA fills, so HAM hits
    2.4GHz before real work; steady-state PE duty is high enough to
    stay warm.
  - x arrives in 6 pieces on one HWDGE queue: the first and last
    groups split into 2-tile pieces (faster pipeline fill / shorter
    drain), middle groups as single 2.1MB DMAs (16KB/partition rows).
  - output via HWDGE (sync) instead of SWDGE: ~2us less tail latency.
PE layout per piece (baseline-proven): stage1 col-tiled token halves
(0,0)/(0,64) sharing W_k; stage2 row-tiled tile pairs (0,0)/(64,0)
K=64; pool 3 softmax groups col-tiled (0,0)/(0,32)/(0,64) into one
shared PSUM accumulator bank (PE-zeroed, all start=False).
PSUM: 2x logits [128,3,512] (3 banks each) + h2 (1) + acc (1) = 8.
"""

import os
import sys

import numpy as np

for _p in ("/opt/trn_rl_repo", os.path.expanduser("~/.axon_site/_ro/trn_rl_repo")):
    if os.path.isdir(_p) and _p not in sys.path:
        sys.path.insert(0, _p)

import concourse.bass as bass
import concourse.mybir as mybir
import concourse.tile as tile
from concourse.bass_utils import run_bass_kernel_spmd


def _ensure_axon_hooks():
    """bass_utils' trace path imports antenv.axon_hooks, which this image's
    antenv stub doesn't ship. Provide it, registering the same ctypes NTFF
    hook the axon boot shim would install when the PJRT .so supports it."""
    try:
        import antenv.axon_hooks  # noqa: F401
        return
    except ImportError:
        pass
    import contextlib
    import ctypes
    import types

    import antenv

    mod = types.ModuleType("antenv.axon_hooks")
    _box = [None]
    mod.set_axon_ntff_profile_hook = lambda h: _box.__setitem__(0, h)
    mod.get_axon_ntff_profile_hook = lambda: _box[0]
    sys.modules["antenv.axon_hooks"] = mod
    antenv.axon_hooks = mod

    so_path = "/opt/axon/libaxon_pjrt.so"
    if not os.path.exists(so_path):
        return
    try:
        lib = ctypes.CDLL(so_path)
    except OSError:
        return
    if not hasattr(lib, "axon_start_nrt_profile"):
        return
    lib.axon_start_nrt_profile.argtypes = [ctypes.POINTER(ctypes.c_int64), ctypes.c_size_t]
    lib.axon_start_nrt_profile.restype = ctypes.c_int64
    lib.axon_stop_nrt_profile.argtypes = [ctypes.c_char_p]
    lib.axon_stop_nrt_profile.restype = ctypes.c_int64

    @contextlib.contextmanager
    def _hook(output_dir, device_ids):
        import jax

        jax.devices()
        if device_ids:
            ids = (ctypes.c_int64 * len(device_ids))(*device_ids)
            rc = lib.axon_start_nrt_profile(ids, len(device_ids))
        else:
            rc = lib.axon_start_nrt_profile(None, 0)
        if rc != 0:
            raise RuntimeError(f"axon_start_nrt_profile rc={rc}")
        try:
            yield
        finally:
            n = lib.axon_stop_nrt_profile(str(output_dir).encode())
            print(f"ntff profile: {n} file(s) written to {output_dir}", file=sys.stderr)

    _box[0] = _hook


B, S, D, DS = 4, 4096, 2048, 64
N_GROUP = 512
N_TOT = 3 * N_GROUP
TOPK_C, TOPK_QK, TOPK_V = 8, 4, 6
N_CORES = 8
S_SH = S // N_CORES          # 512 sequence positions per core
T = B * S_SH                 # 2048 tokens per core
KCH = D // 128               # 16 contraction chunks
N_TTILE = T // 128           # 16 token tiles of 128
F32 = mybir.dt.float32
F16 = mybir.dt.float16

# Pieces: (param_name, [global tile indices], kind). 2-tile pieces carry
# tile A in stage1 col-tile (0,0) -> hT rows 0-63, tile B in (0,64) ->
# rows 64-127; stage2 row-pairs (A, B). 4-tile groups are the baseline
# layout: col A = tiles (4g, 4g+1), col B = (4g+2, 4g+3); stage2 pairs
# (4g, 4g+2) and (4g+1, 4g+3).
PIECES = [
    ("xp0", [0, 2], 2),
    ("xp1", [1, 3], 2),
    ("xg1", [4, 5, 6, 7], 4),
    ("xg2", [8, 9, 10, 11], 4),
    ("xp6", [12, 14], 2),
    ("xp7", [13, 15], 2),
]
N_JUNK = 16                  # PE warm-up matmuls (N=512) while DMA fills

LAST_RESULTS = None  # BassKernelResults of the most recent run (for test harness)


def build_nc():
    nc = bass.Bass()
    xp = {}
    for name, tiles, kind in PIECES:
        xp[name] = nc.declare_dram_parameter(
            name, [128, KCH, 128 * len(tiles)], F16, isOutput=False)
    impT = nc.declare_dram_parameter("impT", [128, N_TTILE], F32, isOutput=False)
    Wt = nc.declare_dram_parameter("Wt", [128, KCH * DS], F16, isOutput=False)
    b2 = nc.declare_dram_parameter("b2", [128, 1], F32, isOutput=False)
    embT2 = nc.declare_dram_parameter("embT2", [128, N_TOT], F16, isOutput=False)
    densep = nc.declare_dram_parameter("densep", [68, N_GROUP], F32, isOutput=True)

    EXPF = mybir.ActivationFunctionType.Exp
    ADD = mybir.AluOpType.add
    MULT = mybir.AluOpType.mult

    with tile.TileContext(nc) as tc:
        with (
            tc.tile_pool(name="consts", bufs=1) as consts,
            tc.tile_pool(name="xin2", bufs=2) as xin2,
            tc.tile_pool(name="xin4", bufs=2) as xin4,
            tc.tile_pool(name="hTp", bufs=2) as hTp,
            tc.tile_pool(name="expp", bufs=4) as expp,
            tc.tile_pool(name="scrp", bufs=2) as scrp,
            tc.tile_pool(name="small", bufs=4) as small,
            tc.tile_pool(name="outp", bufs=1) as outp,
            tc.tile_pool(name="h2_pool", bufs=1, space="PSUM") as h2_pool,
            tc.tile_pool(name="lg_pool", bufs=2, space="PSUM") as lg_pool,
            tc.tile_pool(name="acc_pool", bufs=1, space="PSUM") as acc_pool,
        ):
            # ---- constants: junk/zero tiles first (no DMA), then DMAs in
            # priority order: W -> piece0 -> emb -> b/imp -> rest of x ----
            zw_s = consts.tile([128, 128], F16)
            nc.vector.memset(zw_s, 0.0)
            junk_s = consts.tile([128, N_GROUP], F16)
            nc.vector.memset(junk_s, 0.0)
            cball = consts.tile([128, N_TTILE, 3, 4], F16)
            nc.vector.memset(cball, 0.0)

            w_s = consts.tile([128, KCH * DS], F16)
            nc.sync.dma_start(out=w_s, in_=Wt[:])
            x_s = {}
            name0, tiles0, kind0 = PIECES[0]
            x_s[name0] = xin2.tile([128, KCH, 128 * len(tiles0)], F16,
                                   name=f"x_{name0}", tag="x2", bufs=2)
            nc.sync.dma_start(out=x_s[name0], in_=xp[name0][:])
            embT_s = consts.tile([128, N_TOT], F16)
            nc.sync.dma_start(out=embT_s, in_=embT2[:])
            b_s = consts.tile([128, 1], F32)
            nc.sync.dma_start(out=b_s, in_=b2[:])
            imp_s = consts.tile([128, N_TTILE], F32)
            nc.sync.dma_start(out=imp_s, in_=impT[:])
            # DVE-side absorber: pre-consume the b_s DMA lane into DVE's
            # clock so the per-piece bias-adds only need their PE wait.
            dve_scr = small.tile([128, 1], F32, name="dve_scr", tag="dve_scr",
                                 bufs=1)
            nc.vector.tensor_copy(out=dve_scr, in_=b_s)
            for name, tiles, kind in PIECES[1:]:
                pool = xin2 if kind == 2 else xin4
                x_s[name] = pool.tile([128, KCH, 128 * len(tiles)], F16,
                                      name=f"x_{name}", tag=f"x{kind}", bufs=2)
                nc.sync.dma_start(out=x_s[name], in_=xp[name][:])

            # ---- PSUM accumulator bank for the pool + PE warm-up ----
            acc_t = acc_pool.tile([68, N_GROUP], F32)
            # Junk matmuls: keep the PE busy from t=0 so HAM un-throttles
            # to 2.4GHz before real work. They only depend on the memsets.
            for ji in range(N_JUNK):
                nc.tensor.matmul(
                    acc_t[0:4, :], zw_s[:, 0:4], junk_s[:, :],
                    start=True, stop=False, skip_group_check=True,
                )
            # Zero the acc accumulator rows with the PE (order-safe: zero
            # values make accumulate-vs-overwrite equivalent), start=True
            # on each also resets the bank's has_written from the junk.
            for gi in range(3):
                nc.tensor.matmul(
                    acc_t[32 * gi:32 * gi + 4, :], zw_s[:, 0:4], junk_s[:, :],
                    start=True, stop=False,
                    tile_position=(0, 32 * gi), skip_group_check=True,
                )

            ntile_done = 0
            last_tile = PIECES[-1][1][-1]
            for name, tiles, kind in PIECES:
                xt = x_s[name]
                ntok = 128 * len(tiles)
                half = ntok // 2
                # ---- stage 1: hT [128, ntok/2], col-tiled token halves ----
                h2 = h2_pool.tile([128, 256], F32, name=f"h2_{name}", tag="h2")
                nc.tensor.matmul(h2, zw_s, w_s[:, 0:256], start=True,
                                 stop=False, skip_group_check=True)
                for k in range(KCH):
                    wk = w_s[:, k * DS:(k + 1) * DS]
                    xk = xt[:, k, :]
                    nc.tensor.matmul(
                        h2[0:64, 0:half], wk, xk[:, 0:half],
                        start=False, stop=(k == KCH - 1),
                        tile_position=(0, 0), skip_group_check=True,
                    )
                    nc.tensor.matmul(
                        h2[64:128, 0:half], wk, xk[:, half:ntok],
                        start=False, stop=(k == KCH - 1),
                        tile_position=(0, 64), skip_group_check=True,
                    )
                hT2 = hTp.tile([128, 256], F16, name=f"hT2_{name}", tag="hT2")
                nc.vector.tensor_scalar_add(out=hT2[:, 0:half], in0=h2[:, 0:half],
                                            scalar1=b_s)

                # ---- per row-pair rounds: stage2 + exp + z + c + pool ----
                nround = len(tiles) // 2
                for r in range(nround):
                    jA = tiles[r]
                    jB = tiles[r + nround]
                    lgA = lg_pool.tile([128, 3, N_GROUP], F32,
                                       name=f"lgA_{name}_{r}", tag="lgA", bufs=1)
                    lgB = lg_pool.tile([128, 3, N_GROUP], F32,
                                       name=f"lgB_{name}_{r}", tag="lgB", bufs=1)
                    lhA = hT2[0:64, r * 128:(r + 1) * 128]
                    lhB = hT2[64:128, r * 128:(r + 1) * 128]
                    for gi in range(3):
                        nc.tensor.matmul(
                            lgA[:, gi, :], lhA,
                            embT_s[0:64, gi * N_GROUP:(gi + 1) * N_GROUP],
                            start=True, stop=True, tile_position=(0, 0),
                        )
                        nc.tensor.matmul(
                            lgB[:, gi, :], lhB,
                            embT_s[64:128, gi * N_GROUP:(gi + 1) * N_GROUP],
                            start=True, stop=True, tile_position=(64, 0),
                        )
                    for j, lgt in ((jA, lgA), (jB, lgB)):
                        g = j // 4
                        exp_t = expp.tile([128, 3, N_GROUP], F16,
                                          name=f"exp_{j}", tag="exp")
                        nc.scalar.activation(out=exp_t, in_=lgt, func=EXPF)
                        # z: two fp16 pairwise folds (2x_1p) + fp32 reduce
                        s1 = scrp.tile([128, 3, 256], F16, name=f"s1_{j}",
                                       tag="s1")
                        nc.vector.tensor_tensor(
                            out=s1, in0=exp_t[:, :, 0:256],
                            in1=exp_t[:, :, 256:512], op=ADD)
                        s2 = scrp.tile([128, 3, 128], F16, name=f"s2_{j}",
                                       tag="s2")
                        nc.vector.tensor_tensor(
                            out=s2, in0=s1[:, :, 0:128],
                            in1=s1[:, :, 128:256], op=ADD)
                        zmat = small.tile([128, 3], F32, name=f"z_{j}",
                                          tag=f"z_{j}", bufs=1)
                        nc.vector.tensor_reduce(
                            out=zmat, in_=s2, op=ADD,
                            axis=mybir.AxisListType.X)
                        rz = small.tile([128, 3], F32, name=f"rz_{j}",
                                        tag=f"rz_{j}", bufs=1)
                        nc.vector.reciprocal(out=rz, in_=zmat)
                        nc.vector.tensor_scalar(
                            out=cball[:, j, :, g], in0=rz,
                            scalar1=imp_s[:, j:j + 1], scalar2=0.0,
                            op0=MULT, op1=ADD)
                        for gi in range(3):
                            nc.tensor.matmul(
                                acc_t[32 * gi:32 * gi + 4, :],
                                cball[:, j, gi, :], exp_t[:, gi, :],
                                start=False,
                                stop=(j == last_tile),
                                tile_position=(0, 32 * gi),
                                skip_group_check=True,
                            )
                        ntile_done += 1

            dense_s = outp.tile([68, N_GROUP], F32)
            nc.vector.tensor_copy(out=dense_s, in_=acc_t)
            nc.sync.dma_start(out=densep[:], in_=dense_s)

    _strip_self_waits(nc)
    _strip_dma_waw_waits(nc)
    _slim_tail_drain(nc)
    return nc


def _slim_tail_drain(nc):
    """The TileContext tail drain carries one wait per proc, but the SP
    CTRL_NO lowering has a small wait budget.  Every dependency funnels
    through the output DMA: it waits on the dense_s DVE copy, which waits
    on the final PE stop-matmul, which transitively covers every input
    DMA, ACT and DVE tick.  So the SP drain only needs the output DMA's
    completion -- keep just that lane's wait."""
    blocks = nc.m.functions[0].blocks
    out_lane = None
    for blk in blocks:
        for ins in blk.instructions:
            if type(ins).__name__ == "InstDMACopy" and ins.sync_info:
                for u in ins.sync_info.on_update:
                    if u.ant_name.startswith("DMAHW"):
                        out_lane = u.ant_name
    assert out_lane is not None
    drain = blocks[-1].instructions[0]
    assert type(drain).__name__ == "InstDrain" and drain.sync_info
    waits = list(drain.sync_info.on_wait)
    if len(waits) <= 1:
        return
    keep = [w for w in waits if w.ant_name == out_lane]
    assert len(keep) == 1, f"waits {[w.ant_name for w in waits]} lane {out_lane}"
    drain.sync_info = mybir.SyncInfo(
        on_wait=keep, on_update=list(drain.sync_info.on_update))


_SELF_SEM = {
    "InstMatmult": "PE",
    "InstLdweights": "PE",
    "InstActivation": "Activation",
    "InstTensorScalarPtr": "DVE",
    "InstTensorTensor": "DVE",
    "InstTensorReduce": "DVE",
    "InstTensorCopy": "DVE",
    "InstReciprocal": "DVE",
}


def _merge_same_sem(waits):
    """Collapse waits on the same semaphore to the max threshold."""
    by_name = {}
    for w in waits:
        prev = by_name.get(w.ant_name)
        if prev is None or w.wait_value > prev.wait_value:
            by_name[w.ant_name] = w
    return list(by_name.values())


def _strip_self_waits(nc):
    """Matmul (and several other engine-op) lowerings support a single sync
    wait.  Tile emits a same-engine completion wait for PSUM/SBUF-slot WAW
    reuse on top of the real cross-engine wait; compute engines are strict
    FIFO with in-order completion (PE matmuls are pc-monotone in start AND
    end), so the same-engine wait is hardware-redundant.  Drop it when an
    instruction carries more than one wait.

    Additionally, the exp ACTIVATEs can carry {PE logits RAW, DVE exp-slot
    WAR}.  The DVE WAR is on fold1(j-4), which is older (same DVE FIFO)
    than cball-ts(j-4), which pool-MM(j-4) waited on, and pool-MM(j-4)
    precedes logits-MM(j) in PE program order -- so the instruction's PE
    wait (threshold = logits(j)) transitively covers the DVE WAR.  Drop
    the DVE wait from ACT instructions that also carry a PE wait."""
    for blk in nc.m.functions[0].blocks:
        for ins in blk.instructions:
            nm = type(ins).__name__
            sem = _SELF_SEM.get(nm)
            if sem is None:
                continue
            si = ins.sync_info
            if not si or len(si.on_wait) <= 1:
                continue
            keep = [w for w in si.on_wait if not w.ant_name.startswith(sem)]
            keep = _merge_same_sem(keep)
            if (
                nm == "InstActivation"
                and len(keep) == 2
                and any(w.ant_name.startswith("PE") for w in keep)
                and any(w.ant_name.startswith("DVE") for w in keep)
            ):
                keep = [w for w in keep if w.ant_name.startswith("PE")]
            assert len(keep) <= 1, (
                f"{ins.name}: {len(keep)} cross-engine waits "
                f"{[w.ant_name for w in si.on_wait]}"
            )
            ins.sync_info = mybir.SyncInfo(
                on_wait=keep, on_update=list(si.on_update))


def _strip_dma_waw_waits(nc):
    """Input-DMA rewrites of a rotating SBUF slot carry both the WAR wait on
    the slot's PE readers and a WAW wait on the slot's previous DMA lane.
    The PE readers waited on that DMA lane themselves, so the PE wait
    transitively covers the DMAHW one; DMA_DIRECT2D only has one wait slot."""
    for blk in nc.m.functions[0].blocks:
        for ins in blk.instructions:
            if type(ins).__name__ != "InstDMACopy":
                continue
            si = ins.sync_info
            if not si or len(si.on_wait) <= 1:
                continue
            waits = list(si.on_wait)
            eng = [w for w in waits
                   if w.ant_name.startswith("PE") or w.ant_name.startswith("DVE")]
            if not eng:
                continue
            keep = [w for w in waits if not w.ant_name.startswith("DMAHW")]
            assert len(keep) <= 1, (
                f"{ins.name}: waits {[w.ant_name for w in waits]}"
            )
            ins.sync_info = mybir.SyncInfo(
                on_wait=keep, on_update=list(si.on_update))


def _topk_sparsify(w: np.ndarray, k: int) -> np.ndarray:
    # w [B, N]: keep top-k per row, zero the rest, renormalize.
    idx = np.argpartition(-w, k - 1, axis=-1)[:, :k]
    sparse = np.zeros_like(w)
    np.put_along_axis(sparse, idx, np.take_along_axis(w, idx, axis=-1), axis=-1)
    return sparse / (sparse.sum(axis=-1, keepdims=True) + 1e-8)


def kernel(x, importance, W_proj, b_proj, neuron_emb):
    global LAST_RESULTS
    x = np.asarray(x, dtype=np.float32)
    importance = np.asarray(importance, dtype=np.float32)
    W_proj = np.asarray(W_proj, dtype=np.float32)
    b_proj = np.asarray(b_proj, dtype=np.float32)
    neuron_emb = np.asarray(neuron_emb, dtype=np.float32)

    # Replicated small weights, device-friendly layouts.
    norm = np.maximum(np.linalg.norm(neuron_emb, axis=-1, keepdims=True), 1e-12)
    embT = np.ascontiguousarray((neuron_emb / norm).T)                      # [64, 1536]
    embT2 = np.concatenate([embT, embT], axis=0).astype(np.float16)         # [128, 1536]
    Wt = np.ascontiguousarray(
        W_proj.reshape(KCH, 128, DS).transpose(1, 0, 2).reshape(128, KCH * DS)
    ).astype(np.float16)
    b2 = np.concatenate([b_proj, b_proj]).reshape(128, 1).astype(np.float32)

    in_maps = []
    for c in range(N_CORES):
        xc = x[:, c * S_SH:(c + 1) * S_SH, :].reshape(T, D)     # [2048 t, 2048 d]
        impc = importance[:, c * S_SH:(c + 1) * S_SH].reshape(T)
        impTc = np.ascontiguousarray(impc.reshape(N_TTILE, 128).T)          # [128, 16]
        m = {"impT": impTc, "Wt": Wt, "b2": b2, "embT2": embT2}
        for name, tiles, kind in PIECES:
            tok = np.concatenate([np.arange(128 * j, 128 * (j + 1)) for j in tiles])
            xl = xc[tok]                                        # [L, 2048]
            # -> [128 p, 16 chunk, L tok] with d = chunk*128 + p
            m[name] = np.ascontiguousarray(
                xl.reshape(len(tok), KCH, 128).transpose(2, 1, 0)
            ).astype(np.float16)
        in_maps.append(m)

    _ensure_axon_hooks()
    nc = build_nc()
    try:
        res = run_bass_kernel_spmd(nc, in_maps, core_ids=list(range(N_CORES)))
    except Exception as e:  # trace/profile plumbing can fail; rerun untraced
        if os.environ.get("BASS_NEVER_TRACE") == "1":
            raise
        print(f"traced run failed ({type(e).__name__}: {e}); retrying untraced",
              file=sys.stderr)
        os.environ["BASS_NEVER_TRACE"] = "1"
        try:
            res = run_bass_kernel_spmd(nc, in_maps, core_ids=list(range(N_CORES)))
        finally:
            del os.environ["BASS_NEVER_TRACE"]
    LAST_RESULTS = res
    if getattr(res, "exec_time_ns", None) is not None:
        print(f"HW exec time: {res.exec_time_ns} ns")

    dense = np.zeros((B, N_TOT), dtype=np.float64)
    for r in res.results:
        dp = r["densep"]  # [68, 512]: rows 0-3 = C, 32-35 = QK, 64-67 = V
        dense[:, 0:512] += dp[0:4].astype(np.float64)
        dense[:, 512:1024] += dp[32:36].astype(np.float64)
        dense[:, 1024:1536] += dp[64:68].astype(np.float64)
    dense = dense.astype(np.float32)

    dense_C = dense[:, :N_GROUP]
    dense_QK = dense[:, N_GROUP:2 * N_GROUP]
    dense_V = dense[:, 2 * N_GROUP:]
    w_C = _topk_sparsify(dense_C, TOPK_C)
    w_Q = _topk_sparsify(dense_QK, TOPK_QK)
    w_K = _topk_sparsify(dense_QK, TOPK_QK)
    w_V = _topk_sparsify(dense_V, TOPK_V)
    return np.stack([w_C, w_Q, w_K, w_V], axis=0).astype(np.float32)


# revision 13
# speedup vs baseline: 1.3232x; 1.0374x over previous
"""DAWNBlock MoE-routing kernel for 8 Trainium2 NeuronCores.

Reference computation (shapes hardcoded):
  x [4, 4096, 2048] -> h = x @ W_proj + b_proj          [4, 4096, 64]
  logits = h @ normalize(neuron_emb).T                  [4, 4096, 1536]
  softmax over 3 groups of 512 (C / QK / V)
  dense_g = einsum('bs,bsn->bn', importance, softmax_g) [4, 512] x3
  top-k sparsify + renormalize (k = 8 / 4 / 4 / 6)      -> [4, 4, 512]

Sharding: data-parallel over S (4096 -> 8 x 512). Each core processes
2048 tokens (all 4 batches x its S-slice = 16 tiles of 128 tokens),
producing a partial dense [4, 1536]. Host sums partials + tiny top-k.

v3 design (vs the 80us v2): the v2 critical path was the per-tile
softmax chain exp(ACT 1.5us) -> 3x z-accum(DVE 1x, 2.3us) -> recip ->
c(ACT) -> pool, ~2.7us/tile serial, with the PE stuck at 1.2GHz (HAM
never warmed).  v3:
  - z via two fp16 tensor_tensor pairwise folds (DVE 2x_1p fast mode)
    + one small tensor_reduce: ~1.2us/tile on DVE instead of 2.3, and
    ACT does one plain 1536-wide exp (1.57us) with no accum splits.
  - c = imp * (1/z) moved ACT -> DVE (tiny).
  - PE warm-up: junk matmuls at t=0 while DMA fills, so HAM hits
    2.4GHz before real work; steady-state PE duty is high enough to
    stay warm.
  - x arrives in 6 pieces on one HWDGE queue: the first and last
    groups split into 2-tile pieces (faster pipeline fill / shorter
    drain), middle groups as single 2.1MB DMAs (16KB/partition rows).
  - output via HWDGE (sync) instead of SWDGE: ~2us less tail latency.
PE layout per piece (baseline-proven): stage1 col-tiled token halves
(0,0)/(0,64) sharing W_k; stage2 row-tiled tile pairs (0,0)/(64,0)
K=64; pool 3 softmax groups col-tiled (0,0)/(0,32)/(0,64) into one
shared PSUM accumulator bank (PE-zeroed, all start=False).
PSUM: 2x logits [128,3,512] (3 banks each) + h2 (1) + acc (1) = 8.
"""

import os
import sys

import numpy as np

for _p in ("/opt/trn_rl_repo", os.path.expanduser("~/.axon_site/_ro/trn_rl_repo")):
    if os.path.isdir(_p) and _p not in sys.path:
        sys.path.insert(0, _p)

import concourse.bass as bass
import concourse.mybir as mybir
import concourse.tile as tile
from concourse.bass_utils import run_bass_kernel_spmd


def _ensure_axon_hooks():
    """bass_utils' trace path imports antenv.axon_hooks, which this image's
    antenv stub doesn't ship. Provide it, registering the same ctypes NTFF
    hook the axon boot shim would install when the PJRT .so supports it."""
    try:
        import antenv.axon_hooks  # noqa: F401
        return
    except ImportError:
        pass
    import contextlib
    import ctypes
    import types

    import antenv

    mod = types.ModuleType("antenv.axon_hooks")
    _box = [None]
    mod.set_axon_ntff_profile_hook = lambda h: _box.__setitem__(0, h)
    mod.get_axon_ntff_profile_hook = lambda: _box[0]
    sys.modules["antenv.axon_hooks"] = mod
    antenv.axon_hooks = mod

    so_path = "/opt/axon/libaxon_pjrt.so"
    if not os.path.exists(so_path):
        return
    try:
        lib = ctypes.CDLL(so_path)
    except OSError:
        return
    if not hasattr(lib, "axon_start_nrt_profile"):
        return
    lib.axon_start_nrt_profile.argtypes = [ctypes.POINTER(ctypes.c_int64), ctypes.c_size_t]
    lib.axon_start_nrt_profile.restype = ctypes.c_int64
    lib.axon_stop_nrt_profile.argtypes = [ctypes.c_char_p]
    lib.axon_stop_nrt_profile.restype = ctypes.c_int64

    @contextlib.contextmanager
    def _hook(output_dir, device_ids):
        import jax

        jax.devices()
        if device_ids:
            ids = (ctypes.c_int64 * len(device_ids))(*device_ids)
            rc = lib.axon_start_nrt_profile(ids, len(device_ids))
        else:
            rc = lib.axon_start_nrt_profile(None, 0)
        if rc != 0:
            raise RuntimeError(f"axon_start_nrt_profile rc={rc}")
        try:
            yield
        finally:
            n = lib.axon_stop_nrt_profile(str(output_dir).encode())
            print(f"ntff profile: {n} file(s) written to {output_dir}", file=sys.stderr)

    _box[0] = _hook


B, S, D, DS = 4, 4096, 2048, 64
N_GROUP = 512
N_TOT = 3 * N_GROUP
TOPK_C, TOPK_QK, TOPK_V = 8, 4, 6
N_CORES = 8
S_SH = S // N_CORES          # 512 sequence positions per core
T = B * S_SH                 # 2048 tokens per core
KCH = D // 128               # 16 contraction chunks
N_TTILE = T // 128           # 16 token tiles of 128
F32 = mybir.dt.float32
F16 = mybir.dt.float16

# Pieces: (param_name, [global tile indices], kind). 2-tile pieces carry
# tile A in stage1 col-tile (0,0) -> hT rows 0-63, tile B in (0,64) ->
# rows 64-127; stage2 row-pairs (A, B). 4-tile groups are the baseline
# layout: col A = tiles (4g, 4g+1), col B = (4g+2, 4g+3); stage2 pairs
# (4g, 4g+2) and (4g+1, 4g+3).
PIECES = [
    ("xp0", [0, 2], 2),
    ("xp1", [1, 3], 2),
    ("xg1", [4, 5, 6, 7], 4),
    ("xg2", [8, 9, 10, 11], 4),
    ("xp6", [12, 14], 2),
    ("xp7", [13, 15], 2),
]
N_JUNK = 16                  # PE warm-up matmuls (N=512) while DMA fills

LAST_RESULTS = None  # BassKernelResults of the most recent run (for test harness)


def build_nc():
    nc = bass.Bass()
    xp = {}
    for name, tiles, kind in PIECES:
        xp[name] = nc.declare_dram_parameter(
            name, [128, KCH, 128 * len(tiles)], F16, isOutput=False)
    impT = nc.declare_dram_parameter("impT", [128, N_TTILE], F32, isOutput=False)
    Wt = nc.declare_dram_parameter("Wt", [128, KCH * DS], F16, isOutput=False)
    b2 = nc.declare_dram_parameter("b2", [128, 1], F32, isOutput=False)
    embT2 = nc.declare_dram_parameter("embT2", [128, N_TOT], F16, isOutput=False)
    densep = nc.declare_dram_parameter("densep", [68, N_GROUP], F32, isOutput=True)

    EXPF = mybir.ActivationFunctionType.Exp
    ADD = mybir.AluOpType.add
    MULT = mybir.AluOpType.mult

    with tile.TileContext(nc) as tc:
        with (
            tc.tile_pool(name="consts", bufs=1) as consts,
            tc.tile_pool(name="xin2", bufs=2) as xin2,
            tc.tile_pool(name="xin4", bufs=2) as xin4,
            tc.tile_pool(name="hTp", bufs=2) as hTp,
            tc.tile_pool(name="expp", bufs=6) as expp,
            tc.tile_pool(name="scrp", bufs=2) as scrp,
            tc.tile_pool(name="small", bufs=4) as small,
            tc.tile_pool(name="outp", bufs=1) as outp,
            tc.tile_pool(name="h2_pool", bufs=1, space="PSUM") as h2_pool,
            tc.tile_pool(name="lg_pool", bufs=2, space="PSUM") as lg_pool,
            tc.tile_pool(name="acc_pool", bufs=1, space="PSUM") as acc_pool,
        ):
            # ---- constants: junk/zero tiles first (no DMA), then DMAs in
            # priority order: W -> piece0 -> emb -> b/imp -> rest of x ----
            zw_s = consts.tile([128, 128], F16)
            nc.vector.memset(zw_s, 0.0)
            junk_s = consts.tile([128, N_GROUP], F16)
            nc.vector.memset(junk_s, 0.0)
            cball = consts.tile([128, N_TTILE, 3, 4], F16)
            nc.vector.memset(cball, 0.0)

            w_s = consts.tile([128, KCH * DS], F16)
            nc.sync.dma_start(out=w_s, in_=Wt[:])
            x_s = {}
            name0, tiles0, kind0 = PIECES[0]
            x_s[name0] = xin2.tile([128, KCH, 128 * len(tiles0)], F16,
                                   name=f"x_{name0}", tag="x2", bufs=2)
            nc.sync.dma_start(out=x_s[name0], in_=xp[name0][:])
            embT_s = consts.tile([128, N_TOT], F16)
            nc.sync.dma_start(out=embT_s, in_=embT2[:])
            b_s = consts.tile([128, 1], F32)
            nc.sync.dma_start(out=b_s, in_=b2[:])
            imp_s = consts.tile([128, N_TTILE], F32)
            nc.sync.dma_start(out=imp_s, in_=impT[:])
            # DVE-side absorber: pre-consume the b_s DMA lane into DVE's
            # clock so the per-piece bias-adds only need their PE wait.
            dve_scr = small.tile([128, 1], F32, name="dve_scr", tag="dve_scr",
                                 bufs=1)
            nc.vector.tensor_copy(out=dve_scr, in_=b_s)
            for name, tiles, kind in PIECES[1:]:
                pool = xin2 if kind == 2 else xin4
                x_s[name] = pool.tile([128, KCH, 128 * len(tiles)], F16,
                                      name=f"x_{name}", tag=f"x{kind}", bufs=2)
                nc.sync.dma_start(out=x_s[name], in_=xp[name][:])

            # ---- PSUM accumulator bank for the pool + PE warm-up ----
            acc_t = acc_pool.tile([68, N_GROUP], F32)
            # Junk matmuls: keep the PE busy from t=0 so HAM un-throttles
            # to 2.4GHz before real work. They only depend on the memsets.
            for ji in range(N_JUNK):
                nc.tensor.matmul(
                    acc_t[0:4, :], zw_s[:, 0:4], junk_s[:, :],
                    start=True, stop=False, skip_group_check=True,
                )
            # Zero the acc accumulator rows with the PE (order-safe: zero
            # values make accumulate-vs-overwrite equivalent), start=True
            # on each also resets the bank's has_written from the junk.
            for gi in range(3):
                nc.tensor.matmul(
                    acc_t[32 * gi:32 * gi + 4, :], zw_s[:, 0:4], junk_s[:, :],
                    start=True, stop=False,
                    tile_position=(0, 32 * gi), skip_group_check=True,
                )

            # Pool matmuls are deferred by POOL_LAG rounds: the PE is strict
            # FIFO, so a pool MM (which waits on the z-chain c-weights)
            # placed directly after its own round's stage2 would stall the
            # PE on the ACT/DVE softmax chain every round.  With lag 2 the
            # c-weights are long ready by the time the PE reaches the pool.
            POOL_LAG = 2
            pending_pools = []

            def emit_pools(upto):
                while pending_pools and len(pending_pools) > upto:
                    for emit in pending_pools.pop(0):
                        emit()

            last_tile = PIECES[-1][1][-1]
            for name, tiles, kind in PIECES:
                xt = x_s[name]
                ntok = 128 * len(tiles)
                half = ntok // 2
                # ---- stage 1: hT [128, ntok/2], col-tiled token halves ----
                h2 = h2_pool.tile([128, 256], F32, name=f"h2_{name}", tag="h2")
                nc.tensor.matmul(h2, zw_s, w_s[:, 0:256], start=True,
                                 stop=False, skip_group_check=True)
                for k in range(KCH):
                    wk = w_s[:, k * DS:(k + 1) * DS]
                    xk = xt[:, k, :]
                    nc.tensor.matmul(
                        h2[0:64, 0:half], wk, xk[:, 0:half],
                        start=False, stop=(k == KCH - 1),
                        tile_position=(0, 0), skip_group_check=True,
                    )
                    nc.tensor.matmul(
                        h2[64:128, 0:half], wk, xk[:, half:ntok],
                        start=False, stop=(k == KCH - 1),
                        tile_position=(0, 64), skip_group_check=True,
                    )
                hT2 = hTp.tile([128, 256], F16, name=f"hT2_{name}", tag="hT2")
                nc.vector.tensor_scalar_add(out=hT2[:, 0:half], in0=h2[:, 0:half],
                                            scalar1=b_s)

                # ---- per row-pair rounds: stage2 + exp + z + c + pool ----
                nround = len(tiles) // 2
                for r in range(nround):
                    jA = tiles[r]
                    jB = tiles[r + nround]
                    lgA = lg_pool.tile([128, 3, N_GROUP], F32,
                                       name=f"lgA_{name}_{r}", tag="lgA", bufs=1)
                    lgB = lg_pool.tile([128, 3, N_GROUP], F32,
                                       name=f"lgB_{name}_{r}", tag="lgB", bufs=1)
                    lhA = hT2[0:64, r * 128:(r + 1) * 128]
                    lhB = hT2[64:128, r * 128:(r + 1) * 128]
                    for gi in range(3):
                        nc.tensor.matmul(
                            lgA[:, gi, :], lhA,
                            embT_s[0:64, gi * N_GROUP:(gi + 1) * N_GROUP],
                            start=True, stop=True, tile_position=(0, 0),
                        )
                        nc.tensor.matmul(
                            lgB[:, gi, :], lhB,
                            embT_s[64:128, gi * N_GROUP:(gi + 1) * N_GROUP],
                            start=True, stop=True, tile_position=(64, 0),
                        )
                    # Flush pools that are POOL_LAG rounds old.  This must
                    # happen BEFORE this round's exp ops: an exp reusing an
                    # SBUF slot must come after that slot's pool reader in
                    # program order (expp bufs=6 = 3 rounds of slack > lag 2).
                    emit_pools(POOL_LAG - 1)
                    round_pools = []
                    for j, lgt in ((jA, lgA), (jB, lgB)):
                        g = j // 4
                        exp_t = expp.tile([128, 3, N_GROUP], F16,
                                          name=f"exp_{j}", tag="exp")
                        nc.scalar.activation(out=exp_t, in_=lgt, func=EXPF)
                        # z: two fp16 pairwise folds (2x_1p) + fp32 reduce
                        s1 = scrp.tile([128, 3, 256], F16, name=f"s1_{j}",
                                       tag="s1")
                        nc.vector.tensor_tensor(
                            out=s1, in0=exp_t[:, :, 0:256],
                            in1=exp_t[:, :, 256:512], op=ADD)
                        s2 = scrp.tile([128, 3, 128], F16, name=f"s2_{j}",
                                       tag="s2")
                        nc.vector.tensor_tensor(
                            out=s2, in0=s1[:, :, 0:128],
                            in1=s1[:, :, 128:256], op=ADD)
                        zmat = small.tile([128, 3], F32, name=f"z_{j}",
                                          tag=f"z_{j}", bufs=1)
                        nc.vector.tensor_reduce(
                            out=zmat, in_=s2, op=ADD,
                            axis=mybir.AxisListType.X)
                        rz = small.tile([128, 3], F32, name=f"rz_{j}",
                                        tag=f"rz_{j}", bufs=1)
                        nc.vector.reciprocal(out=rz, in_=zmat)
                        nc.vector.tensor_scalar(
                            out=cball[:, j, :, g], in0=rz,
                            scalar1=imp_s[:, j:j + 1], scalar2=0.0,
                            op0=MULT, op1=ADD)

                        def mk_pool(j=j, exp_t=exp_t):
                            def emit():
                                for gi in range(3):
                                    nc.tensor.matmul(
                                        acc_t[32 * gi:32 * gi + 4, :],
                                        cball[:, j, gi, :], exp_t[:, gi, :],
                                        start=False,
                                        stop=(j == last_tile),
                                        tile_position=(0, 32 * gi),
                                        skip_group_check=True,
                                    )
                            return emit
                        round_pools.append(mk_pool())
                    pending_pools.append(round_pools)
            emit_pools(0)

            dense_s = outp.tile([68, N_GROUP], F32)
            nc.vector.tensor_copy(out=dense_s, in_=acc_t)
            nc.sync.dma_start(out=densep[:], in_=dense_s)

    _strip_self_waits(nc)
    _strip_dma_waw_waits(nc)
    _slim_tail_drain(nc)
    return nc


def _slim_tail_drain(nc):
    """The TileContext tail drain carries one wait per proc, but the SP
    CTRL_NO lowering has a small wait budget.  Every dependency funnels
    through the output DMA: it waits on the dense_s DVE copy, which waits
    on the final PE stop-matmul, which transitively covers every input
    DMA, ACT and DVE tick.  So the SP drain only needs the output DMA's
    completion -- keep just that lane's wait."""
    blocks = nc.m.functions[0].blocks
    out_lane = None
    for blk in blocks:
        for ins in blk.instructions:
            if type(ins).__name__ == "InstDMACopy" and ins.sync_info:
                for u in ins.sync_info.on_update:
                    if u.ant_name.startswith("DMAHW"):
                        out_lane = u.ant_name
    assert out_lane is not None
    drain = blocks[-1].instructions[0]
    assert type(drain).__name__ == "InstDrain" and drain.sync_info
    waits = list(drain.sync_info.on_wait)
    if len(waits) <= 1:
        return
    keep = [w for w in waits if w.ant_name == out_lane]
    assert len(keep) == 1, f"waits {[w.ant_name for w in waits]} lane {out_lane}"
    drain.sync_info = mybir.SyncInfo(
        on_wait=keep, on_update=list(drain.sync_info.on_update))


_SELF_SEM = {
    "InstMatmult": "PE",
    "InstLdweights": "PE",
    "InstActivation": "Activation",
    "InstTensorScalarPtr": "DVE",
    "InstTensorTensor": "DVE",
    "InstTensorReduce": "DVE",
    "InstTensorCopy": "DVE",
    "InstReciprocal": "DVE",
}


def _merge_same_sem(waits):
    """Collapse waits on the same semaphore to the max threshold."""
    by_name = {}
    for w in waits:
        prev = by_name.get(w.ant_name)
        if prev is None or w.wait_value > prev.wait_value:
            by_name[w.ant_name] = w
    return list(by_name.values())


def _strip_self_waits(nc):
    """Matmul (and several other engine-op) lowerings support a single sync
    wait.  Tile emits a same-engine completion wait for PSUM/SBUF-slot WAW
    reuse on top of the real cross-engine wait; compute engines are strict
    FIFO with in-order completion (PE matmuls are pc-monotone in start AND
    end), so the same-engine wait is hardware-redundant.  Drop it when an
    instruction carries more than one wait.

    Additionally, the exp ACTIVATEs can carry {PE logits RAW, DVE exp-slot
    WAR}.  The DVE WAR is on fold1(j-4), which is older (same DVE FIFO)
    than cball-ts(j-4), which pool-MM(j-4) waited on, and pool-MM(j-4)
    precedes logits-MM(j) in PE program order -- so the instruction's PE
    wait (threshold = logits(j)) transitively covers the DVE WAR.  Drop
    the DVE wait from ACT instructions that also carry a PE wait."""
    for blk in nc.m.functions[0].blocks:
        for ins in blk.instructions:
            nm = type(ins).__name__
            sem = _SELF_SEM.get(nm)
            if sem is None:
                continue
            si = ins.sync_info
            if not si or len(si.on_wait) <= 1:
                continue
            keep = [w for w in si.on_wait if not w.ant_name.startswith(sem)]
            keep = _merge_same_sem(keep)
            if (
                nm == "InstActivation"
                and len(keep) == 2
                and any(w.ant_name.startswith("PE") for w in keep)
                and any(w.ant_name.startswith("DVE") for w in keep)
            ):
                keep = [w for w in keep if w.ant_name.startswith("PE")]
            assert len(keep) <= 1, (
                f"{ins.name}: {len(keep)} cross-engine waits "
                f"{[w.ant_name for w in si.on_wait]}"
            )
            ins.sync_info = mybir.SyncInfo(
                on_wait=keep, on_update=list(si.on_update))


def _strip_dma_waw_waits(nc):
    """Input-DMA rewrites of a rotating SBUF slot carry both the WAR wait on
    the slot's PE readers and a WAW wait on the slot's previous DMA lane.
    The PE readers waited on that DMA lane themselves, so the PE wait
    transitively covers the DMAHW one; DMA_DIRECT2D only has one wait slot."""
    for blk in nc.m.functions[0].blocks:
        for ins in blk.instructions:
            if type(ins).__name__ != "InstDMACopy":
                continue
            si = ins.sync_info
            if not si or len(si.on_wait) <= 1:
                continue
            waits = list(si.on_wait)
            eng = [w for w in waits
                   if w.ant_name.startswith("PE") or w.ant_name.startswith("DVE")]
            if not eng:
                continue
            keep = [w for w in waits if not w.ant_name.startswith("DMAHW")]
            assert len(keep) <= 1, (
                f"{ins.name}: waits {[w.ant_name for w in waits]}"
            )
            ins.sync_info = mybir.SyncInfo(
                on_wait=keep, on_update=list(si.on_update))


def _topk_sparsify(w: np.ndarray, k: int) -> np.ndarray:
    # w [B, N]: keep top-k per row, zero the rest, renormalize.
    idx = np.argpartition(-w, k - 1, axis=-1)[:, :k]
    sparse = np.zeros_like(w)
    np.put_along_axis(sparse, idx, np.take_along_axis(w, idx, axis=-1), axis=-1)
    return sparse / (sparse.sum(axis=-1, keepdims=True) + 1e-8)


def kernel(x, importance, W_proj, b_proj, neuron_emb):
    global LAST_RESULTS
    x = np.asarray(x, dtype=np.float32)
    importance = np.asarray(importance, dtype=np.float32)
    W_proj = np.asarray(W_proj, dtype=np.float32)
    b_proj = np.asarray(b_proj, dtype=np.float32)
    neuron_emb = np.asarray(neuron_emb, dtype=np.float32)

    # Replicated small weights, device-friendly layouts.
    norm = np.maximum(np.linalg.norm(neuron_emb, axis=-1, keepdims=True), 1e-12)
    embT = np.ascontiguousarray((neuron_emb / norm).T)                      # [64, 1536]
    embT2 = np.concatenate([embT, embT], axis=0).astype(np.float16)         # [128, 1536]
    Wt = np.ascontiguousarray(
        W_proj.reshape(KCH, 128, DS).transpose(1, 0, 2).reshape(128, KCH * DS)
    ).astype(np.float16)
    b2 = np.concatenate([b_proj, b_proj]).reshape(128, 1).astype(np.float32)

    in_maps = []
    for c in range(N_CORES):
        xc = x[:, c * S_SH:(c + 1) * S_SH, :].reshape(T, D)     # [2048 t, 2048 d]
        impc = importance[:, c * S_SH:(c + 1) * S_SH].reshape(T)
        impTc = np.ascontiguousarray(impc.reshape(N_TTILE, 128).T)          # [128, 16]
        m = {"impT": impTc, "Wt": Wt, "b2": b2, "embT2": embT2}
        for name, tiles, kind in PIECES:
            tok = np.concatenate([np.arange(128 * j, 128 * (j + 1)) for j in tiles])
            xl = xc[tok]                                        # [L, 2048]
            # -> [128 p, 16 chunk, L tok] with d = chunk*128 + p
            m[name] = np.ascontiguousarray(
                xl.reshape(len(tok), KCH, 128).transpose(2, 1, 0)
            ).astype(np.float16)
        in_maps.append(m)

    _ensure_axon_hooks()
    nc = build_nc()
    try:
        res = run_bass_kernel_spmd(nc, in_maps, core_ids=list(range(N_CORES)))
    except Exception as e:  # trace/profile plumbing can fail; rerun untraced
        if os.environ.get("BASS_NEVER_TRACE") == "1":
            raise
        print(f"traced run failed ({type(e).__name__}: {e}); retrying untraced",
              file=sys.stderr)
        os.environ["BASS_NEVER_TRACE"] = "1"
        try:
            res = run_bass_kernel_spmd(nc, in_maps, core_ids=list(range(N_CORES)))
        finally:
            del os.environ["BASS_NEVER_TRACE"]
    LAST_RESULTS = res
    if getattr(res, "exec_time_ns", None) is not None:
        print(f"HW exec time: {res.exec_time_ns} ns")

    dense = np.zeros((B, N_TOT), dtype=np.float64)
    for r in res.results:
        dp = r["densep"]  # [68, 512]: rows 0-3 = C, 32-35 = QK, 64-67 = V
        dense[:, 0:512] += dp[0:4].astype(np.float64)
        dense[:, 512:1024] += dp[32:36].astype(np.float64)
        dense[:, 1024:1536] += dp[64:68].astype(np.float64)
    dense = dense.astype(np.float32)

    dense_C = dense[:, :N_GROUP]
    dense_QK = dense[:, N_GROUP:2 * N_GROUP]
    dense_V = dense[:, 2 * N_GROUP:]
    w_C = _topk_sparsify(dense_C, TOPK_C)
    w_Q = _topk_sparsify(dense_QK, TOPK_QK)
    w_K = _topk_sparsify(dense_QK, TOPK_QK)
    w_V = _topk_sparsify(dense_V, TOPK_V)
    return np.stack([w_C, w_Q, w_K, w_V], axis=0).astype(np.float32)


# revision 15
# speedup vs baseline: 1.3360x; 1.0096x over previous
"""DAWNBlock MoE-routing kernel for 8 Trainium2 NeuronCores.

Reference computation (shapes hardcoded):
  x [4, 4096, 2048] -> h = x @ W_proj + b_proj          [4, 4096, 64]
  logits = h @ normalize(neuron_emb).T                  [4, 4096, 1536]
  softmax over 3 groups of 512 (C / QK / V)
  dense_g = einsum('bs,bsn->bn', importance, softmax_g) [4, 512] x3
  top-k sparsify + renormalize (k = 8 / 4 / 4 / 6)      -> [4, 4, 512]

Sharding: data-parallel over S (4096 -> 8 x 512). Each core processes
2048 tokens (all 4 batches x its S-slice = 16 tiles of 128 tokens),
producing a partial dense [4, 1536]. Host sums partials + tiny top-k.

v3 design (vs the 80us v2): the v2 critical path was the per-tile
softmax chain exp(ACT 1.5us) -> 3x z-accum(DVE 1x, 2.3us) -> recip ->
c(ACT) -> pool, ~2.7us/tile serial, with the PE stuck at 1.2GHz (HAM
never warmed).  v3:
  - z via two fp16 tensor_tensor pairwise folds (DVE 2x_1p fast mode)
    + one small tensor_reduce: ~1.2us/tile on DVE instead of 2.3, and
    ACT does one plain 1536-wide exp (1.57us) with no accum splits.
  - c = imp * (1/z) moved ACT -> DVE (tiny).
  - PE warm-up: junk matmuls at t=0 while DMA fills, so HAM hits
    2.4GHz before real work; steady-state PE duty is high enough to
    stay warm.
  - x arrives in 6 pieces on one HWDGE queue: the first and last
    groups split into 2-tile pieces (faster pipeline fill / shorter
    drain), middle groups as single 2.1MB DMAs (16KB/partition rows).
  - output via HWDGE (sync) instead of SWDGE: ~2us less tail latency.
PE layout per piece (baseline-proven): stage1 col-tiled token halves
(0,0)/(0,64) sharing W_k; stage2 row-tiled tile pairs (0,0)/(64,0)
K=64; pool 3 softmax groups col-tiled (0,0)/(0,32)/(0,64) into one
shared PSUM accumulator bank (PE-zeroed, all start=False).
PSUM: 2x logits [128,3,512] (3 banks each) + h2 (1) + acc (1) = 8.
"""

import os
import sys

import numpy as np

for _p in ("/opt/trn_rl_repo", os.path.expanduser("~/.axon_site/_ro/trn_rl_repo")):
    if os.path.isdir(_p) and _p not in sys.path:
        sys.path.insert(0, _p)

import concourse.bass as bass
import concourse.mybir as mybir
import concourse.tile as tile
from concourse.bass_utils import run_bass_kernel_spmd


def _ensure_axon_hooks():
    """bass_utils' trace path imports antenv.axon_hooks, which this image's
    antenv stub doesn't ship. Provide it, registering the same ctypes NTFF
    hook the axon boot shim would install when the PJRT .so supports it."""
    try:
        import antenv.axon_hooks  # noqa: F401
        return
    except ImportError:
        pass
    import contextlib
    import ctypes
    import types

    import antenv

    mod = types.ModuleType("antenv.axon_hooks")
    _box = [None]
    mod.set_axon_ntff_profile_hook = lambda h: _box.__setitem__(0, h)
    mod.get_axon_ntff_profile_hook = lambda: _box[0]
    sys.modules["antenv.axon_hooks"] = mod
    antenv.axon_hooks = mod

    so_path = "/opt/axon/libaxon_pjrt.so"
    if not os.path.exists(so_path):
        return
    try:
        lib = ctypes.CDLL(so_path)
    except OSError:
        return
    if not hasattr(lib, "axon_start_nrt_profile"):
        return
    lib.axon_start_nrt_profile.argtypes = [ctypes.POINTER(ctypes.c_int64), ctypes.c_size_t]
    lib.axon_start_nrt_profile.restype = ctypes.c_int64
    lib.axon_stop_nrt_profile.argtypes = [ctypes.c_char_p]
    lib.axon_stop_nrt_profile.restype = ctypes.c_int64

    @contextlib.contextmanager
    def _hook(output_dir, device_ids):
        import jax

        jax.devices()
        if device_ids:
            ids = (ctypes.c_int64 * len(device_ids))(*device_ids)
            rc = lib.axon_start_nrt_profile(ids, len(device_ids))
        else:
            rc = lib.axon_start_nrt_profile(None, 0)
        if rc != 0:
            raise RuntimeError(f"axon_start_nrt_profile rc={rc}")
        try:
            yield
        finally:
            n = lib.axon_stop_nrt_profile(str(output_dir).encode())
            print(f"ntff profile: {n} file(s) written to {output_dir}", file=sys.stderr)

    _box[0] = _hook


B, S, D, DS = 4, 4096, 2048, 64
N_GROUP = 512
N_TOT = 3 * N_GROUP
TOPK_C, TOPK_QK, TOPK_V = 8, 4, 6
N_CORES = 8
S_SH = S // N_CORES          # 512 sequence positions per core
T = B * S_SH                 # 2048 tokens per core
KCH = D // 128               # 16 contraction chunks
N_TTILE = T // 128           # 16 token tiles of 128
F32 = mybir.dt.float32
F16 = mybir.dt.float16

# Pieces: (param_name, [global tile indices], kind). 2-tile pieces carry
# tile A in stage1 col-tile (0,0) -> hT rows 0-63, tile B in (0,64) ->
# rows 64-127; stage2 row-pairs (A, B). 4-tile groups are the baseline
# layout: col A = tiles (4g, 4g+1), col B = (4g+2, 4g+3); stage2 pairs
# (4g, 4g+2) and (4g+1, 4g+3).
PIECES = [
    ("xp0", [0, 2], 2),
    ("xp1", [1, 3], 2),
    ("xg1", [4, 5, 6, 7], 4),
    ("xg2", [8, 9, 10, 11], 4),
    ("xp6", [12, 14], 2),
    ("xp7", [13, 15], 2),
]
N_JUNK = 16                  # PE warm-up matmuls (N=512) while DMA fills

LAST_RESULTS = None  # BassKernelResults of the most recent run (for test harness)


def build_nc():
    nc = bass.Bass()
    xp = {}
    for name, tiles, kind in PIECES:
        xp[name] = nc.declare_dram_parameter(
            name, [128, KCH, 128 * len(tiles)], F16, isOutput=False)
    impT = nc.declare_dram_parameter("impT", [128, N_TTILE], F32, isOutput=False)
    Wt = nc.declare_dram_parameter("Wt", [128, KCH * DS], F16, isOutput=False)
    b2 = nc.declare_dram_parameter("b2", [128, 1], F32, isOutput=False)
    embT2 = nc.declare_dram_parameter("embT2", [128, N_TOT], F16, isOutput=False)
    densep = nc.declare_dram_parameter("densep", [68, N_GROUP], F32, isOutput=True)

    EXPF = mybir.ActivationFunctionType.Exp
    ADD = mybir.AluOpType.add
    MULT = mybir.AluOpType.mult

    with tile.TileContext(nc) as tc:
        with (
            tc.tile_pool(name="consts", bufs=1) as consts,
            tc.tile_pool(name="xin2", bufs=2) as xin2,
            tc.tile_pool(name="xin4", bufs=2) as xin4,
            tc.tile_pool(name="hTp", bufs=2) as hTp,
            tc.tile_pool(name="expp", bufs=6) as expp,
            tc.tile_pool(name="scrp", bufs=2) as scrp,
            tc.tile_pool(name="small", bufs=4) as small,
            tc.tile_pool(name="outp", bufs=1) as outp,
            tc.tile_pool(name="h2_pool", bufs=1, space="PSUM") as h2_pool,
            tc.tile_pool(name="lg_pool", bufs=2, space="PSUM") as lg_pool,
            tc.tile_pool(name="acc_pool", bufs=1, space="PSUM") as acc_pool,
        ):
            # ---- constants: junk/zero tiles first (no DMA), then DMAs in
            # priority order: W -> piece0 -> emb -> b/imp -> rest of x ----
            zw_s = consts.tile([128, 128], F16)
            nc.vector.memset(zw_s, 0.0)
            junk_s = consts.tile([128, N_GROUP], F16)
            nc.vector.memset(junk_s, 0.0)
            cball = consts.tile([128, N_TTILE, 3, 4], F16)
            nc.gpsimd.memset(cball, 0.0)

            w_s = consts.tile([128, KCH * DS], F16)
            nc.sync.dma_start(out=w_s, in_=Wt[:])
            x_s = {}
            name0, tiles0, kind0 = PIECES[0]
            x_s[name0] = xin2.tile([128, KCH, 128 * len(tiles0)], F16,
                                   name=f"x_{name0}", tag="x2", bufs=2)
            nc.sync.dma_start(out=x_s[name0], in_=xp[name0][:])
            embT_s = consts.tile([128, N_TOT], F16)
            nc.sync.dma_start(out=embT_s, in_=embT2[:])
            b_s = consts.tile([128, 1], F32)
            nc.sync.dma_start(out=b_s, in_=b2[:])
            imp_s = consts.tile([128, N_TTILE], F32)
            nc.sync.dma_start(out=imp_s, in_=impT[:])
            # DVE-side absorber: pre-consume the b_s DMA lane into DVE's
            # clock so the per-piece bias-adds only need their PE wait.
            dve_scr = small.tile([128, 1], F32, name="dve_scr", tag="dve_scr",
                                 bufs=1)
            nc.vector.tensor_copy(out=dve_scr, in_=b_s)
            # GPSIMD-side absorber for the imp DMA lane (read by the
            # per-tile c-weight op which runs on GPSIMD).
            gp_scr = small.tile([128, 1], F32, name="gp_scr", tag="gp_scr",
                                bufs=1)
            nc.gpsimd.tensor_copy(out=gp_scr, in_=imp_s[:, 0:1])
            for name, tiles, kind in PIECES[1:]:
                pool = xin2 if kind == 2 else xin4
                x_s[name] = pool.tile([128, KCH, 128 * len(tiles)], F16,
                                      name=f"x_{name}", tag=f"x{kind}", bufs=2)
                nc.sync.dma_start(out=x_s[name], in_=xp[name][:])

            # ---- PSUM accumulator bank for the pool + PE warm-up ----
            acc_t = acc_pool.tile([68, N_GROUP], F32)
            # Junk matmuls: keep the PE busy from t=0 so HAM un-throttles
            # to 2.4GHz before real work. They only depend on the memsets.
            for ji in range(N_JUNK):
                nc.tensor.matmul(
                    acc_t[0:4, :], zw_s[:, 0:4], junk_s[:, :],
                    start=True, stop=False, skip_group_check=True,
                )
            # Zero the acc accumulator rows with the PE (order-safe: zero
            # values make accumulate-vs-overwrite equivalent), start=True
            # on each also resets the bank's has_written from the junk.
            for gi in range(3):
                nc.tensor.matmul(
                    acc_t[32 * gi:32 * gi + 4, :], zw_s[:, 0:4], junk_s[:, :],
                    start=True, stop=False,
                    tile_position=(0, 32 * gi), skip_group_check=True,
                )

            # Pool matmuls are deferred by POOL_LAG rounds: the PE is strict
            # FIFO, so a pool MM (which waits on the z-chain c-weights)
            # placed directly after its own round's stage2 would stall the
            # PE on the ACT/DVE softmax chain every round.  With lag 2 the
            # c-weights are long ready by the time the PE reaches the pool.
            POOL_LAG = 2
            pending_pools = []

            def emit_pools(upto):
                while pending_pools and len(pending_pools) > upto:
                    for emit in pending_pools.pop(0):
                        emit()

            last_tile = PIECES[-1][1][-1]
            for name, tiles, kind in PIECES:
                xt = x_s[name]
                ntok = 128 * len(tiles)
                half = ntok // 2
                # ---- stage 1: hT [128, ntok/2], col-tiled token halves ----
                h2 = h2_pool.tile([128, 256], F32, name=f"h2_{name}", tag="h2")
                nc.tensor.matmul(h2, zw_s, w_s[:, 0:256], start=True,
                                 stop=False, skip_group_check=True)
                for k in range(KCH):
                    wk = w_s[:, k * DS:(k + 1) * DS]
                    xk = xt[:, k, :]
                    nc.tensor.matmul(
                        h2[0:64, 0:half], wk, xk[:, 0:half],
                        start=False, stop=(k == KCH - 1),
                        tile_position=(0, 0), skip_group_check=True,
                    )
                    nc.tensor.matmul(
                        h2[64:128, 0:half], wk, xk[:, half:ntok],
                        start=False, stop=(k == KCH - 1),
                        tile_position=(0, 64), skip_group_check=True,
                    )
                hT2 = hTp.tile([128, 256], F16, name=f"hT2_{name}", tag="hT2")
                nc.vector.tensor_scalar_add(out=hT2[:, 0:half], in0=h2[:, 0:half],
                                            scalar1=b_s)

                # ---- per row-pair rounds: stage2 + exp + z + c + pool ----
                nround = len(tiles) // 2
                for r in range(nround):
                    jA = tiles[r]
                    jB = tiles[r + nround]
                    lgA = lg_pool.tile([128, 3, N_GROUP], F32,
                                       name=f"lgA_{name}_{r}", tag="lgA", bufs=1)
                    lgB = lg_pool.tile([128, 3, N_GROUP], F32,
                                       name=f"lgB_{name}_{r}", tag="lgB", bufs=1)
                    lhA = hT2[0:64, r * 128:(r + 1) * 128]
                    lhB = hT2[64:128, r * 128:(r + 1) * 128]
                    for gi in range(3):
                        nc.tensor.matmul(
                            lgA[:, gi, :], lhA,
                            embT_s[0:64, gi * N_GROUP:(gi + 1) * N_GROUP],
                            start=True, stop=True, tile_position=(0, 0),
                        )
                        nc.tensor.matmul(
                            lgB[:, gi, :], lhB,
                            embT_s[64:128, gi * N_GROUP:(gi + 1) * N_GROUP],
                            start=True, stop=True, tile_position=(64, 0),
                        )
                    # Flush pools that are POOL_LAG rounds old.  This must
                    # happen BEFORE this round's exp ops: an exp reusing an
                    # SBUF slot must come after that slot's pool reader in
                    # program order (expp bufs=6 = 3 rounds of slack > lag 2).
                    emit_pools(POOL_LAG - 1)
                    round_pools = []
                    for j, lgt in ((jA, lgA), (jB, lgB)):
                        g = j // 4
                        exp_t = expp.tile([128, 3, N_GROUP], F16,
                                          name=f"exp_{j}", tag="exp")
                        nc.scalar.activation(out=exp_t, in_=lgt, func=EXPF)
                        # z: two fp16 pairwise folds (2x_1p) + fp32 reduce
                        s1 = scrp.tile([128, 3, 256], F16, name=f"s1_{j}",
                                       tag="s1")
                        nc.vector.tensor_tensor(
                            out=s1, in0=exp_t[:, :, 0:256],
                            in1=exp_t[:, :, 256:512], op=ADD)
                        s2 = scrp.tile([128, 3, 128], F16, name=f"s2_{j}",
                                       tag="s2")
                        nc.vector.tensor_tensor(
                            out=s2, in0=s1[:, :, 0:128],
                            in1=s1[:, :, 128:256], op=ADD)
                        zmat = small.tile([128, 3], F32, name=f"z_{j}",
                                          tag=f"z_{j}", bufs=1)
                        nc.vector.tensor_reduce(
                            out=zmat, in_=s2, op=ADD,
                            axis=mybir.AxisListType.X)
                        rz = small.tile([128, 3], F32, name=f"rz_{j}",
                                        tag=f"rz_{j}", bufs=1)
                        nc.vector.reciprocal(out=rz, in_=zmat)
                        nc.gpsimd.tensor_scalar(
                            out=cball[:, j, :, g], in0=rz,
                            scalar1=imp_s[:, j:j + 1], scalar2=0.0,
                            op0=MULT, op1=ADD)

                        def mk_pool(j=j, exp_t=exp_t):
                            def emit():
                                for gi in range(3):
                                    nc.tensor.matmul(
                                        acc_t[32 * gi:32 * gi + 4, :],
                                        cball[:, j, gi, :], exp_t[:, gi, :],
                                        start=False,
                                        stop=(j == last_tile),
                                        tile_position=(0, 32 * gi),
                                        skip_group_check=True,
                                    )
                            return emit
                        round_pools.append(mk_pool())
                    pending_pools.append(round_pools)
            emit_pools(0)

            dense_s = outp.tile([68, N_GROUP], F32)
            nc.vector.tensor_copy(out=dense_s, in_=acc_t)
            nc.sync.dma_start(out=densep[:], in_=dense_s)

    _strip_self_waits(nc)
    _strip_dma_waw_waits(nc)
    _slim_tail_drain(nc)
    return nc


def _slim_tail_drain(nc):
    """The TileContext tail drain carries one wait per proc, but the SP
    CTRL_NO lowering has a small wait budget.  Every dependency funnels
    through the output DMA: it waits on the dense_s DVE copy, which waits
    on the final PE stop-matmul, which transitively covers every input
    DMA, ACT and DVE tick.  So the SP drain only needs the output DMA's
    completion -- keep just that lane's wait."""
    blocks = nc.m.functions[0].blocks
    out_lane = None
    for blk in blocks:
        for ins in blk.instructions:
            if type(ins).__name__ == "InstDMACopy" and ins.sync_info:
                for u in ins.sync_info.on_update:
                    if u.ant_name.startswith("DMAHW"):
                        out_lane = u.ant_name
    assert out_lane is not None
    drain = blocks[-1].instructions[0]
    assert type(drain).__name__ == "InstDrain" and drain.sync_info
    waits = list(drain.sync_info.on_wait)
    if len(waits) <= 1:
        return
    keep = [w for w in waits if w.ant_name == out_lane]
    assert len(keep) == 1, f"waits {[w.ant_name for w in waits]} lane {out_lane}"
    drain.sync_info = mybir.SyncInfo(
        on_wait=keep, on_update=list(drain.sync_info.on_update))


_SELF_SEM = {
    "InstMatmult": "PE",
    "InstLdweights": "PE",
    "InstActivation": "Activation",
    "InstTensorScalarPtr": "DVE",
    "InstTensorTensor": "DVE",
    "InstTensorReduce": "DVE",
    "InstTensorCopy": "DVE",
    "InstReciprocal": "DVE",
}


def _merge_same_sem(waits):
    """Collapse waits on the same semaphore to the max threshold."""
    by_name = {}
    for w in waits:
        prev = by_name.get(w.ant_name)
        if prev is None or w.wait_value > prev.wait_value:
            by_name[w.ant_name] = w
    return list(by_name.values())


def _strip_self_waits(nc):
    """Matmul (and several other engine-op) lowerings support a single sync
    wait.  Tile emits a same-engine completion wait for PSUM/SBUF-slot WAW
    reuse on top of the real cross-engine wait; compute engines are strict
    FIFO with in-order completion (PE matmuls are pc-monotone in start AND
    end), so the same-engine wait is hardware-redundant.  Drop it when an
    instruction carries more than one wait.

    Additionally, the exp ACTIVATEs can carry {PE logits RAW, DVE exp-slot
    WAR}.  The DVE WAR is on fold1(j-4), which is older (same DVE FIFO)
    than cball-ts(j-4), which pool-MM(j-4) waited on, and pool-MM(j-4)
    precedes logits-MM(j) in PE program order -- so the instruction's PE
    wait (threshold = logits(j)) transitively covers the DVE WAR.  Drop
    the DVE wait from ACT instructions that also carry a PE wait."""
    eng_sem = {
        mybir.EngineType.PE: "PE",
        mybir.EngineType.Activation: "Activation",
        mybir.EngineType.DVE: "DVE",
        mybir.EngineType.Pool: "Pool",
    }
    for blk in nc.m.functions[0].blocks:
        for ins in blk.instructions:
            nm = type(ins).__name__
            if nm not in _SELF_SEM:
                continue
            sem = eng_sem.get(ins.engine)
            if sem is None:
                continue
            si = ins.sync_info
            if not si or len(si.on_wait) <= 1:
                continue
            keep = [w for w in si.on_wait if not w.ant_name.startswith(sem)]
            keep = _merge_same_sem(keep)
            if (
                nm == "InstActivation"
                and len(keep) == 2
                and any(w.ant_name.startswith("PE") for w in keep)
                and any(w.ant_name.startswith("DVE") for w in keep)
            ):
                keep = [w for w in keep if w.ant_name.startswith("PE")]
            assert len(keep) <= 1, (
                f"{ins.name}: {len(keep)} cross-engine waits "
                f"{[w.ant_name for w in si.on_wait]}"
            )
            ins.sync_info = mybir.SyncInfo(
                on_wait=keep, on_update=list(si.on_update))


def _strip_dma_waw_waits(nc):
    """Input-DMA rewrites of a rotating SBUF slot carry both the WAR wait on
    the slot's PE readers and a WAW wait on the slot's previous DMA lane.
    The PE readers waited on that DMA lane themselves, so the PE wait
    transitively covers the DMAHW one; DMA_DIRECT2D only has one wait slot."""
    for blk in nc.m.functions[0].blocks:
        for ins in blk.instructions:
            if type(ins).__name__ != "InstDMACopy":
                continue
            si = ins.sync_info
            if not si or len(si.on_wait) <= 1:
                continue
            waits = list(si.on_wait)
            eng = [w for w in waits
                   if w.ant_name.startswith("PE") or w.ant_name.startswith("DVE")]
            if not eng:
                continue
            keep = [w for w in waits if not w.ant_name.startswith("DMAHW")]
            assert len(keep) <= 1, (
                f"{ins.name}: waits {[w.ant_name for w in waits]}"
            )
            ins.sync_info = mybir.SyncInfo(
                on_wait=keep, on_update=list(si.on_update))


def _topk_sparsify(w: np.ndarray, k: int) -> np.ndarray:
    # w [B, N]: keep top-k per row, zero the rest, renormalize.
    idx = np.argpartition(-w, k - 1, axis=-1)[:, :k]
    sparse = np.zeros_like(w)
    np.put_along_axis(sparse, idx, np.take_along_axis(w, idx, axis=-1), axis=-1)
    return sparse / (sparse.sum(axis=-1, keepdims=True) + 1e-8)


def kernel(x, importance, W_proj, b_proj, neuron_emb):
    global LAST_RESULTS
    x = np.asarray(x, dtype=np.float32)
    importance = np.asarray(importance, dtype=np.float32)
    W_proj = np.asarray(W_proj, dtype=np.float32)
    b_proj = np.asarray(b_proj, dtype=np.float32)
    neuron_emb = np.asarray(neuron_emb, dtype=np.float32)

    # Replicated small weights, device-friendly layouts.
    norm = np.maximum(np.linalg.norm(neuron_emb, axis=-1, keepdims=True), 1e-12)
    embT = np.ascontiguousarray((neuron_emb / norm).T)                      # [64, 1536]
    embT2 = np.concatenate([embT, embT], axis=0).astype(np.float16)         # [128, 1536]
    Wt = np.ascontiguousarray(
        W_proj.reshape(KCH, 128, DS).transpose(1, 0, 2).reshape(128, KCH * DS)
    ).astype(np.float16)
    b2 = np.concatenate([b_proj, b_proj]).reshape(128, 1).astype(np.float32)

    in_maps = []
    for c in range(N_CORES):
        xc = x[:, c * S_SH:(c + 1) * S_SH, :].reshape(T, D)     # [2048 t, 2048 d]
        impc = importance[:, c * S_SH:(c + 1) * S_SH].reshape(T)
        impTc = np.ascontiguousarray(impc.reshape(N_TTILE, 128).T)          # [128, 16]
        m = {"impT": impTc, "Wt": Wt, "b2": b2, "embT2": embT2}
        for name, tiles, kind in PIECES:
            tok = np.concatenate([np.arange(128 * j, 128 * (j + 1)) for j in tiles])
            xl = xc[tok]                                        # [L, 2048]
            # -> [128 p, 16 chunk, L tok] with d = chunk*128 + p
            m[name] = np.ascontiguousarray(
                xl.reshape(len(tok), KCH, 128).transpose(2, 1, 0)
            ).astype(np.float16)
        in_maps.append(m)

    _ensure_axon_hooks()
    nc = build_nc()
    try:
        res = run_bass_kernel_spmd(nc, in_maps, core_ids=list(range(N_CORES)))
    except Exception as e:  # trace/profile plumbing can fail; rerun untraced
        if os.environ.get("BASS_NEVER_TRACE") == "1":
            raise
        print(f"traced run failed ({type(e).__name__}: {e}); retrying untraced",
              file=sys.stderr)
        os.environ["BASS_NEVER_TRACE"] = "1"
        try:
            res = run_bass_kernel_spmd(nc, in_maps, core_ids=list(range(N_CORES)))
        finally:
            del os.environ["BASS_NEVER_TRACE"]
    LAST_RESULTS = res
    if getattr(res, "exec_time_ns", None) is not None:
        print(f"HW exec time: {res.exec_time_ns} ns")

    dense = np.zeros((B, N_TOT), dtype=np.float64)
    for r in res.results:
        dp = r["densep"]  # [68, 512]: rows 0-3 = C, 32-35 = QK, 64-67 = V
        dense[:, 0:512] += dp[0:4].astype(np.float64)
        dense[:, 512:1024] += dp[32:36].astype(np.float64)
        dense[:, 1024:1536] += dp[64:68].astype(np.float64)
    dense = dense.astype(np.float32)

    dense_C = dense[:, :N_GROUP]
    dense_QK = dense[:, N_GROUP:2 * N_GROUP]
    dense_V = dense[:, 2 * N_GROUP:]
    w_C = _topk_sparsify(dense_C, TOPK_C)
    w_Q = _topk_sparsify(dense_QK, TOPK_QK)
    w_K = _topk_sparsify(dense_QK, TOPK_QK)
    w_V = _topk_sparsify(dense_V, TOPK_V)
    return np.stack([w_C, w_Q, w_K, w_V], axis=0).astype(np.float32)
